# revision 5
# baseline (speedup 1.0000x reference)
"""Multi-sense skip-gram (MSSG) loss kernel for Trainium2.

Data-parallel over batch across 8 cores; tables packed row-wise into one
[50000, 2100] bf16 table: row v = [global(300) | emb senses(900) | disamb
senses(900)].

Structure (per 128-element tile, 4 tiles/core; HW device time ~278us):
- 16 single-index indirect gathers with flat 2D SBUF dests. Multi-index
  gathers, 3D unit-dim dest views, and SBUF->SBUF accum_op DMAs all pass
  CoreSim but CRASH/corrupt real trn2 HW - do not reintroduce them.
- Dot products: broadcast bf16 tensor_tensor multiplies (DVE 2x mode),
  emitted in HALVES with the first fold level split to match, so folds
  on the first half overlap the second half's multiply (shortens the
  serial disamb critical path; -5us vs unsplit).
- Weighted sums: 30x tensor_scalar_mul (f32 scalar APs - bf16 scalars
  are rejected by the API) + fold tree on DVE.
- pos/neg dot reductions on the Scalar (ACT) engine via per-segment
  Copy+accum_out; sigmoids via the Exp table (exp/+1/reciprocal) and
  loss Lns deferred to after the tile loop to minimize ~1.3us ACT
  activation-table reloads.
"""

import numpy as np

NUM_SENSE = 3
EMB_DIM = 300
VOCAB = 50000
BATCH = 4096
CTX = 10
NEG = 5
N_CORES = 8
P = 128
PER_CORE = BATCH // N_CORES  # 512
TILES = PER_CORE // P        # 4
D = EMB_DIM
CS = CTX * NUM_SENSE         # 30
SN = NUM_SENSE * NEG         # 15
RowLen = D + 2 * NUM_SENSE * D  # 2100: [glob | emb | dis]
EMB_OFF = D                  # 300
DIS_OFF = D + NUM_SENSE * D  # 1200

_CACHE = {}


def _build_bass():
    key = "nc"
    if key in _CACHE:
        return _CACHE[key]

    import concourse.bass as bass
    import concourse.bacc as bacc
    import concourse.tile as tile
    from concourse import mybir

    F32 = mybir.dt.float32
    BF16 = mybir.dt.bfloat16
    I32 = mybir.dt.int32
    AX = mybir.AxisListType
    OP = mybir.AluOpType
    AF = mybir.ActivationFunctionType
    TINY = float(np.finfo(np.float32).tiny)

    nc = bacc.Bacc("TRN2", target_bir_lowering=False, debug=False)

    packed = nc.dram_tensor("packed", [VOCAB, RowLen], BF16, kind="ExternalInput")
    # idx columns: 0..9 ctx, 10 word, 11..15 neg
    idx = nc.dram_tensor("idx", [PER_CORE, 16], I32, kind="ExternalInput")
    out_d = nc.dram_tensor("out", [1, 1], F32, kind="ExternalOutput")

    def tt(out, a, b, op=OP.add):
        nc.vector.tensor_tensor(out=out, in0=a, in1=b, op=op)

    with tile.TileContext(nc) as tc:
        with (
            tc.tile_pool(name="gather", bufs=2) as gp,
            tc.tile_pool(name="tmpp", bufs=2) as tp,
            tc.tile_pool(name="small", bufs=2) as sp,
            tc.tile_pool(name="persist", bufs=1) as pp,
            tc.tile_pool(name="psum", bufs=1, space="PSUM") as psp,
        ):
            acc = pp.tile([P, 2 * TILES], F32)
            ones = pp.tile([P, 1], F32)
            WSAVE = pp.tile([P, TILES * CTX], F32)
            WNSAVE = pp.tile([P, TILES * NEG], F32)
            nc.vector.memset(ones[:], 1.0)

            for t in range(TILES):
                rows = slice(t * P, (t + 1) * P)
                ix = gp.tile([P, 16], I32)
                nc.sync.dma_start(out=ix[:], in_=idx[rows, :])

                PK = gp.tile([P, 11 * RowLen], BF16)   # 10 ctx + word, full rows
                NG = gp.tile([P, NEG * D], BF16)       # neg: glob part only
                PK3 = PK[:].rearrange("p (k x) -> p k x", x=RowLen)
                NG3 = NG[:].rearrange("p (n d) -> p n d", d=D)

                def gather1(dst, offs):
                    nc.gpsimd.indirect_dma_start(
                        out=dst, out_offset=None, in_=packed[:],
                        in_offset=bass.IndirectOffsetOnAxis(ap=offs, axis=0),
                    )

                for k in range(11):
                    gather1(PK[:, k * RowLen:(k + 1) * RowLen], ix[:, k:k + 1])
                for k in range(NEG):
                    gather1(NG[:, k * D:(k + 1) * D], ix[:, 11 + k:12 + k])

                CT3 = PK3[:, 0:CTX, 0:D]                               # [P,10,300]
                AS4 = PK3[:, 0:CTX, EMB_OFF:DIS_OFF].rearrange(
                    "p c (s d) -> p c s d", d=D)                       # [P,10,3,300]
                AD4 = PK3[:, 0:CTX, DIS_OFF:RowLen].rearrange(
                    "p c (s d) -> p c s d", d=D)
                SEN3 = PK3[:, CTX, EMB_OFF:DIS_OFF].rearrange(
                    "p (s d) -> p s d", d=D)                           # [P,3,300]
                DIS3 = PK3[:, CTX, DIS_OFF:RowLen].rearrange(
                    "p (s d) -> p s d", d=D)

                TMP = tp.tile([P, CS * D], BF16, tag="tmp_d")
                T3 = TMP[:].rearrange("p (k d) -> p k d", d=D)
                TMPQ = tp.tile([P, CS * D], BF16, tag="tmp_pn")
                Q3 = TMPQ[:].rearrange("p (k d) -> p k d", d=D)
                SCR = [sp.tile([P, D], BF16, tag=f"scr{i}", name=f"scr{i}")
                       for i in range(4)]

                def fold_reduce_30(src3, zout):
                    """src3 [P,30,300] bf16 products -> zout [P,30] f32 sums."""
                    tt(src3[:, :, 0:150], src3[:, :, 0:150], src3[:, :, 150:300])
                    tt(src3[:, :, 0:74], src3[:, :, 0:74], src3[:, :, 76:150])
                    tt(src3[:, :, 0:38], src3[:, :, 0:38], src3[:, :, 38:76])
                    nc.vector.tensor_reduce(
                        out=zout, in_=src3[:, :, 0:38], axis=AX.X, op=OP.add)

                def act_reduce(src3, zout, nseg):
                    """per-segment [P,300] Copy+accum on ACT -> zout[:, k]."""
                    for k in range(nseg):
                        nc.scalar.activation(
                            out=SCR[k % 4][:], in_=src3[:, k, :], func=AF.Copy,
                            accum_out=zout[:, k:k + 1])

                # ---- ctx1 = sum_c CT (mean deferred via exp scale) ----
                c1a = sp.tile([P, 5 * D], BF16)
                c1b = sp.tile([P, 2 * D], BF16)
                ctx1 = sp.tile([P, D], BF16)
                c1a3 = c1a[:].rearrange("p (c d) -> p c d", d=D)
                tt(c1a3, CT3[:, 0:5, :], CT3[:, 5:10, :])
                tt(c1b[:], c1a[:, 0:2 * D], c1a[:, 2 * D:4 * D])
                tt(c1b[:, 0:D], c1b[:, 0:D], c1b[:, D:2 * D])
                tt(ctx1[:], c1b[:, 0:D], c1a[:, 4 * D:5 * D])

                def disamb_step(ctx_vec, ctx_out, si):
                    # products + fold1 split in halves: folds on the first
                    # half start while the second half still multiplies
                    T4 = T3.rearrange("p (c s) d -> p c s d", s=NUM_SENSE)
                    cb = ctx_vec[:].unsqueeze(1).unsqueeze(1) \
                        .to_broadcast([P, 5, NUM_SENSE, D])
                    tt(T4[:, 0:5], AD4[:, 0:5], cb, OP.mult)
                    tt(T4[:, 5:10], AD4[:, 5:10], cb, OP.mult)
                    z = sp.tile([P, CS], F32, tag=f"z{si}")
                    tt(T3[:, 0:15, 0:150], T3[:, 0:15, 0:150],
                       T3[:, 0:15, 150:300])
                    tt(T3[:, 15:30, 0:150], T3[:, 15:30, 0:150],
                       T3[:, 15:30, 150:300])
                    tt(T3[:, :, 0:74], T3[:, :, 0:74], T3[:, :, 76:150])
                    tt(T3[:, :, 0:38], T3[:, :, 0:38], T3[:, :, 38:76])
                    nc.vector.tensor_reduce(
                        out=z[:], in_=T3[:, :, 0:38], axis=AX.X, op=OP.add)
                    # softmax over s (ACT exp, DVE small ops)
                    E = sp.tile([P, CS], F32, tag=f"E{si}")
                    nc.scalar.activation(out=E[:], in_=z[:], func=AF.Exp,
                                         scale=1.0 / CTX)
                    S = sp.tile([P, CTX], F32, tag=f"S{si}")
                    nc.vector.tensor_reduce(
                        out=S[:], in_=E[:].rearrange("p (c s) -> p c s",
                                                     s=NUM_SENSE),
                        axis=AX.X, op=OP.add)
                    R = sp.tile([P, CTX], F32, tag=f"R{si}")
                    nc.vector.reciprocal(R[:], S[:])
                    AL = sp.tile([P, CS], F32, tag=f"AL{si}")
                    tt(AL[:].rearrange("p (c s) -> p c s", s=NUM_SENSE),
                       E[:].rearrange("p (c s) -> p c s", s=NUM_SENSE),
                       R[:].unsqueeze(2).to_broadcast([P, CTX, NUM_SENSE]),
                       OP.mult)
                    # weighted sum: 30x tensor_scalar (4x mode) + fold tree
                    for k in range(CS):
                        nc.vector.tensor_scalar_mul(
                            out=T3[:, k, :],
                            in0=AS4[:, k // NUM_SENSE, k % NUM_SENSE, :],
                            scalar1=AL[:, k:k + 1])
                    tt(T3[:, 0:15, :], T3[:, 0:15, :], T3[:, 15:30, :])
                    tt(T3[:, 0:7, :], T3[:, 0:7, :], T3[:, 7:14, :])
                    tt(T3[:, 0:3, :], T3[:, 0:3, :], T3[:, 3:6, :])
                    tt(T3[:, 0:1, :], T3[:, 0:1, :], T3[:, 1:2, :])
                    tt(T3[:, 0:1, :], T3[:, 0:1, :], T3[:, 14:15, :])
                    tt(T3[:, 2:3, :], T3[:, 2:3, :], T3[:, 6:7, :])
                    tt(ctx_out[:].unsqueeze(1), T3[:, 0:1, :], T3[:, 2:3, :])

                ctx2 = sp.tile([P, D], BF16)
                ctx3 = sp.tile([P, D], BF16)
                disamb_step(ctx1, ctx2, 0)
                disamb_step(ctx2, ctx3, 1)

                # ---- alpha = softmax_s(DIS . ctx3 / CTX) ----
                tt(T3[:, 0:NUM_SENSE, :], DIS3,
                   ctx3[:].unsqueeze(1).to_broadcast([P, NUM_SENSE, D]),
                   OP.mult)
                av = sp.tile([P, NUM_SENSE], F32)
                a3 = T3[:, 0:NUM_SENSE, :]
                tt(a3[:, :, 0:150], a3[:, :, 0:150], a3[:, :, 150:300])
                tt(a3[:, :, 0:74], a3[:, :, 0:74], a3[:, :, 76:150])
                tt(a3[:, :, 0:38], a3[:, :, 0:38], a3[:, :, 38:76])
                nc.vector.tensor_reduce(
                    out=av[:], in_=a3[:, :, 0:38], axis=AX.X, op=OP.add)
                Ea = sp.tile([P, NUM_SENSE], F32)
                Sa = sp.tile([P, 1], F32)
                nc.scalar.activation(out=Ea[:], in_=av[:], func=AF.Exp,
                                     scale=1.0 / CTX, accum_out=Sa[:, 0:1])
                Ra = sp.tile([P, 1], F32)
                nc.vector.reciprocal(Ra[:], Sa[:])
                ALS = sp.tile([P, NUM_SENSE], F32)
                nc.vector.tensor_scalar_mul(out=ALS[:], in0=Ea[:],
                                            scalar1=Ra[:, 0:1])

                # ---- pos: sigmoid(SEN . CT) over (s,c) ----
                tt(Q3.rearrange("p (s c) d -> p s c d", c=CTX),
                   CT3.unsqueeze(1).to_broadcast([P, NUM_SENSE, CTX, D]),
                   SEN3.unsqueeze(2).to_broadcast([P, NUM_SENSE, CTX, D]),
                   OP.mult)
                PL = sp.tile([P, CS], F32)
                act_reduce(Q3, PL[:], CS)
                PP = sp.tile([P, CS], F32)
                nc.scalar.activation(out=PP[:], in_=PL[:], func=AF.Exp,
                                     scale=-1.0)
                nc.vector.tensor_scalar_add(out=PP[:], in0=PP[:], scalar1=1.0)
                nc.vector.reciprocal(PP[:], PP[:])

                W = sp.tile([P, CS], F32)
                for s in range(NUM_SENSE):
                    nc.vector.tensor_scalar_mul(
                        out=W[:, s * CTX:(s + 1) * CTX],
                        in0=PP[:, s * CTX:(s + 1) * CTX],
                        scalar1=ALS[:, s:s + 1])
                tt(W[:, 0:CTX], W[:, 0:CTX], W[:, CTX:2 * CTX])
                tt(W[:, 0:CTX], W[:, 0:CTX], W[:, 2 * CTX:3 * CTX])
                nc.vector.tensor_scalar_max(out=W[:, 0:CTX], in0=W[:, 0:CTX],
                                            scalar1=TINY)
                nc.vector.tensor_copy(out=WSAVE[:, t * CTX:(t + 1) * CTX],
                                      in_=W[:, 0:CTX])

                # ---- neg: sigmoid(SEN . NG) over (s,n) ----
                tt(Q3[:, 0:SN, :].rearrange("p (s n) d -> p s n d", n=NEG),
                   NG3.unsqueeze(1).to_broadcast([P, NUM_SENSE, NEG, D]),
                   SEN3.unsqueeze(2).to_broadcast([P, NUM_SENSE, NEG, D]),
                   OP.mult)
                NL = sp.tile([P, SN], F32)
                act_reduce(Q3, NL[:], SN)
                NP = sp.tile([P, SN], F32)
                nc.scalar.activation(out=NP[:], in_=NL[:], func=AF.Exp,
                                     scale=-1.0)
                nc.vector.tensor_scalar_add(out=NP[:], in0=NP[:], scalar1=1.0)
                nc.vector.reciprocal(NP[:], NP[:])

                Wn = sp.tile([P, SN], F32)
                for s in range(NUM_SENSE):
                    nc.vector.tensor_scalar_mul(
                        out=Wn[:, s * NEG:(s + 1) * NEG],
                        in0=NP[:, s * NEG:(s + 1) * NEG],
                        scalar1=ALS[:, s:s + 1])
                tt(Wn[:, 0:NEG], Wn[:, 0:NEG], Wn[:, NEG:2 * NEG])
                tt(Wn[:, 0:NEG], Wn[:, 0:NEG], Wn[:, 2 * NEG:3 * NEG])
                # 1 - x, clamp, ln
                nc.vector.tensor_scalar(
                    out=Wn[:, NEG:2 * NEG], in0=Wn[:, 0:NEG],
                    scalar1=-1.0, scalar2=1.0, op0=OP.mult, op1=OP.add)
                nc.vector.tensor_scalar_max(
                    out=Wn[:, NEG:2 * NEG], in0=Wn[:, NEG:2 * NEG], scalar1=TINY)
                nc.vector.tensor_copy(
                    out=WNSAVE[:, t * NEG:(t + 1) * NEG],
                    in_=Wn[:, NEG:2 * NEG])

            WL = pp.tile([P, TILES * CTX], F32)
            WLn = pp.tile([P, TILES * NEG], F32)
            for t in range(TILES):
                nc.scalar.activation(
                    out=WL[:, t * CTX:(t + 1) * CTX],
                    in_=WSAVE[:, t * CTX:(t + 1) * CTX], func=AF.Ln,
                    accum_out=acc[:, 2 * t:2 * t + 1])
            for t in range(TILES):
                nc.scalar.activation(
                    out=WLn[:, t * NEG:(t + 1) * NEG],
                    in_=WNSAVE[:, t * NEG:(t + 1) * NEG], func=AF.Ln,
                    accum_out=acc[:, 2 * t + 1:2 * t + 2])
            total = pp.tile([P, 1], F32)
            nc.vector.tensor_reduce(out=total[:], in_=acc[:], axis=AX.X, op=OP.add)
            ps = psp.tile([1, 1], F32)
            nc.tensor.matmul(out=ps[:], lhsT=total[:], rhs=ones[:],
                             start=True, stop=True)
            res = pp.tile([1, 1], F32)
            nc.scalar.copy(res[:], ps[:])
            nc.sync.dma_start(out=out_d[:, :], in_=res[:])

    nc.compile()
    _CACHE[key] = nc
    return nc


def _prep_inputs(word_ids, context_ids, neg_ids,
                 emb_weight, global_emb_weight, disamb_weight):
    import ml_dtypes
    BF = ml_dtypes.bfloat16
    word_ids = np.asarray(word_ids).astype(np.int32).reshape(BATCH)
    context_ids = np.asarray(context_ids).astype(np.int32).reshape(BATCH, CTX)
    neg_ids = np.asarray(neg_ids).astype(np.int32).reshape(BATCH, NEG)
    packed = np.empty((VOCAB, RowLen), dtype=BF)
    packed[:, 0:D] = np.asarray(
        global_emb_weight, dtype=np.float32).astype(BF).reshape(VOCAB, D)
    packed[:, EMB_OFF:DIS_OFF] = np.asarray(
        emb_weight, dtype=np.float32).astype(BF).reshape(VOCAB, NUM_SENSE * D)
    packed[:, DIS_OFF:RowLen] = np.asarray(
        disamb_weight, dtype=np.float32).astype(BF).reshape(VOCAB, NUM_SENSE * D)
    # idx columns: 0..9 ctx, 10 word, 11..15 neg
    idx_all = np.concatenate(
        [context_ids, word_ids[:, None], neg_ids], axis=1).astype(np.int32)
    in_maps = []
    for c in range(N_CORES):
        sl = slice(c * PER_CORE, (c + 1) * PER_CORE)
        in_maps.append({
            "packed": packed,
            "idx": np.ascontiguousarray(idx_all[sl]),
        })
    return in_maps


def kernel(word_ids, context_ids, context_masks, neg_ids,
           emb_weight, global_emb_weight, disamb_weight):
    from concourse import bass_utils
    nc = _build_bass()
    in_maps = _prep_inputs(word_ids, context_ids, neg_ids,
                           emb_weight, global_emb_weight, disamb_weight)
    res = bass_utils.run_bass_kernel_spmd(nc, in_maps, core_ids=list(range(N_CORES)))
    total = 0.0
    for r in res.results:
        total += float(np.asarray(r["out"]).reshape(-1)[0])
    loss = -total / float(BATCH * CTX)
    return np.array(loss, dtype=np.float32)
